# revision 30
# baseline (speedup 1.0000x reference)
"""BitLinearx (BitNet-style ternary-weight + int8-activation linear) on 8 TRN2 cores.

Tensor-parallel: shard weight rows (out_features=11008) across 8 cores (1376 each),
replicate activations. Each core computes its [8192, 1376] output slice; host
concatenates. s_w = 1/mean(|w|) uses an on-device AllReduce.

Math notes:
- q = clip(round(x * 127/amax), -128, 127) are exact integers in [-128, 127] and
  tw = clip(round(w*s_w), -1, 1) in {-1, 0, 1}: both exact in bf16, and the
  matmul accumulates integers < 2^24 in fp32 PSUM, so the core matmul is EXACT.
- round-to-nearest-even is implemented as (v + 1.5*2^23) - 1.5*2^23 in fp32.
- For the weights, clip-then-round == round-then-clip (clip bounds are integers),
  which lets the two clip ops fuse with the scale multiply.
- For activations round(x * 127/amax) can never leave [-127, 127] (the max
  |x| element maps to exactly +-127), so the activation clip is a no-op and
  is elided. (Only exception: all |x| <= 1e-5, impossible for randn fill.)
"""

import numpy as np

T = 8192
D_IN = 4096
D_OUT = 11008
N_CORES = 8
O_SHARD = D_OUT // N_CORES  # 1376
P = 128
KT = D_IN // P  # 32 contraction tiles
TT = T // P  # 64 token tiles
OT_FULL = O_SHARD // P  # 10 full o tiles
O_REM = O_SHARD - OT_FULL * P  # 96
MAGIC = 12582912.0  # 1.5 * 2**23, fp32 RNE rounding constant
N_CHUNKS = ((0, 512), (512, 512), (1024, 352))  # psum accumulation regions
TPACK = 4  # transposes packed per psum bank before one copyback

_BUILT = None


def _build(n_ttiles=TT, n_repeat=1):
    import concourse.bacc as bacc
    import concourse.mybir as mybir
    import concourse.tile as tile
    from concourse.masks import make_identity

    f32 = mybir.dt.float32
    bf16 = mybir.dt.bfloat16
    AX = mybir.AxisListType
    OP = mybir.AluOpType
    ACTF = mybir.ActivationFunctionType

    # Bacc (not Bass): its finalize() runs generate_event_semaphores /
    # move_matmul_waits_to_ldweights, legalizing multi-wait instructions for
    # walrus (which allows only one sync wait per instruction).
    nc = bacc.Bacc("TRN2", num_devices=N_CORES, num_swdge_queues=4)

    t_rows = n_ttiles * P
    x_d = nc.dram_tensor("x", [t_rows, D_IN], f32, kind="ExternalInput")
    w_d = nc.dram_tensor("w", [O_SHARD, D_IN], f32, kind="ExternalInput")
    out_d = nc.dram_tensor("out", [t_rows, O_SHARD], f32, kind="ExternalOutput")
    cc_in = nc.dram_tensor("cc_in", [P, 1], f32)
    cc_out = nc.dram_tensor("cc_out", [P, 1], f32, addr_space="Shared")

    with tile.TileContext(nc) as tc:
        with (
            tc.tile_pool(name="xw", bufs=3) as xw_pool,  # [128,4096] f32 x/w tiles
            tc.tile_pool(name="qb", bufs=2) as qb_pool,  # [128,4096] bf16 q natural
            tc.tile_pool(name="qt", bufs=2) as qt_pool,  # [128,32,128] bf16 qT
            tc.tile_pool(name="twt", bufs=1) as twt_pool,  # [128,32,1376] bf16 twT
            tc.tile_pool(name="osb", bufs=2) as out_pool,  # [128,1376] f32 out stage
            tc.tile_pool(name="const", bufs=1) as const_pool,
            tc.tile_pool(name="sv", bufs=3) as sv_pool,  # per-tile scalars
            tc.tile_pool(name="pacc", bufs=2, space="PSUM") as pacc,
            tc.tile_pool(name="ptr", bufs=2, space="PSUM") as ptr,
        ):
            # ---------------- constants ----------------
            ones = const_pool.tile([P, P], f32, name="ones")
            nc.gpsimd.memset(ones[:], 1.0)
            ident_b = const_pool.tile([P, P], bf16, name="ident_b")
            make_identity(nc, ident_b[:])
            ident_f = const_pool.tile([P, P], f32, name="ident_f")
            make_identity(nc, ident_f[:])
            m_ap = const_pool.tile([P, 1], f32, name="m_ap")
            nc.gpsimd.memset(m_ap[:], MAGIC)
            zero_ap = const_pool.tile([P, 1], f32, name="zero_ap")
            nc.gpsimd.memset(zero_ap[:], 0.0)

            # ---------------- phase W1: sum(|w|) partials + AllReduce ----------------
            n_wt = OT_FULL + 1
            parts = const_pool.tile([P, n_wt], f32, name="parts")
            nc.vector.memset(parts[:], 0.0)
            for i in range(n_wt):
                rows = P if i < OT_FULL else O_REM
                wt = xw_pool.tile([P, D_IN], f32, tag="xw", name=f"w1_{i}")
                q4 = D_IN // 4
                nc.sync.dma_start(wt[:rows, :q4], w_d[i * P : i * P + rows, :q4])
                nc.scalar.dma_start(
                    wt[:rows, q4 : 2 * q4], w_d[i * P : i * P + rows, q4 : 2 * q4]
                )
                nc.gpsimd.dma_start(
                    wt[:rows, 2 * q4 : 3 * q4],
                    w_d[i * P : i * P + rows, 2 * q4 : 3 * q4],
                )
                nc.gpsimd.dma_start(
                    wt[:rows, 3 * q4 :], w_d[i * P : i * P + rows, 3 * q4 :]
                )
                nc.vector.reduce_sum(
                    parts[:rows, i : i + 1],
                    wt[:rows, :],
                    axis=AX.X,
                    apply_absolute_value=True,
                )
            acc_sum = const_pool.tile([P, 1], f32, name="acc_sum")
            nc.vector.reduce_sum(acc_sum[:], parts[:], axis=AX.X)
            nc.sync.dma_start(cc_in[:], acc_sum[:])
            nc.gpsimd.collective_compute(
                "AllReduce",
                OP.add,
                replica_groups=[list(range(N_CORES))],
                ins=[cc_in[:]],
                outs=[cc_out[:]],
            )
            allred_sb = const_pool.tile([P, 1], f32, name="allred_sb")
            nc.sync.dma_start(allred_sb[:], cc_out[:])

            # cross-partition sum + broadcast in one matmul: psum[m,0]=sum_p allred[p]
            gsum_ps = ptr.tile([P, 1], f32, tag="tr", name="gsum_ps")
            nc.tensor.matmul(gsum_ps[:], ones[:], allred_sb[:], start=True, stop=True)
            mean_c = const_pool.tile([P, 1], f32, name="mean_c")
            nc.vector.tensor_scalar(
                mean_c[:],
                gsum_ps[:],
                1.0 / float(D_OUT * D_IN),
                1e-5,
                op0=OP.mult,
                op1=OP.max,
            )
            s_w = const_pool.tile([P, 1], f32, name="s_w")
            nc.vector.reciprocal(s_w[:], mean_c[:])
            s_w127 = const_pool.tile([P, 1], f32, name="s_w127")
            nc.vector.tensor_scalar(s_w127[:], s_w[:], 1.0 / 127.0, None, op0=OP.mult)

            # ---------------- phase W2: ternarize + transpose w ----------------
            twt = twt_pool.tile([P, KT, O_SHARD], bf16, name="twt")
            for i in range(OT_FULL + 1):
                rows = P if i < OT_FULL else O_REM
                wt = xw_pool.tile([P, D_IN], f32, tag="xw", name=f"w2_{i}")
                q4 = D_IN // 4
                nc.sync.dma_start(wt[:rows, :q4], w_d[i * P : i * P + rows, :q4])
                nc.scalar.dma_start(
                    wt[:rows, q4 : 2 * q4], w_d[i * P : i * P + rows, q4 : 2 * q4]
                )
                nc.gpsimd.dma_start(
                    wt[:rows, 2 * q4 : 3 * q4],
                    w_d[i * P : i * P + rows, 2 * q4 : 3 * q4],
                )
                nc.gpsimd.dma_start(
                    wt[:rows, 3 * q4 :], w_d[i * P : i * P + rows, 3 * q4 :]
                )
                # clamp(w*s_w, -1, 1) then +MAGIC (rounds): two fused in-place passes
                nc.vector.tensor_scalar(
                    wt[:rows, :], wt[:rows, :], s_w[:rows, :], 1.0,
                    op0=OP.mult, op1=OP.min,
                )
                nc.vector.tensor_scalar(
                    wt[:rows, :], wt[:rows, :], -1.0, MAGIC,
                    op0=OP.max, op1=OP.add,
                )
                pst = None
                for k in range(KT):
                    j = k % TPACK
                    if j == 0:
                        pst = ptr.tile(
                            [P, TPACK, P], f32, tag="tr", name=f"wtr_{i}_{k}"
                        )
                    nc.tensor.transpose(
                        pst[:, j, :rows],
                        wt[:rows, k * P : (k + 1) * P],
                        ident_f[:rows, :rows],
                    )
                    if j == TPACK - 1:
                        k0 = k - (TPACK - 1)
                        nc.vector.tensor_scalar(
                            twt[:, k0 : k + 1, i * P : i * P + rows],
                            pst[:, :, :rows],
                            MAGIC,
                            None,
                            op0=OP.subtract,
                        )

            # ---------------- main loop over token tiles ----------------
            for rep, t in ((r, t) for r in range(n_repeat) for t in range(n_ttiles)):
                t = t if rep == 0 else t  # same data each repeat (timing builds)
                sfx = f"{t}" if n_repeat == 1 else f"{rep}_{t}"
                xt = xw_pool.tile([P, D_IN], f32, tag="xw", name=f"x_{sfx}")
                # Spread the 2MB tile load across all DMA paths: each HWDGE
                # ring streams ~22.5GB/s, so one dma_start (89us) would starve
                # the 21us/tile PE pipeline. SP + ACT rings take a quarter
                # each, SWDGE (8 queues) takes the rest.
                q4 = D_IN // 4
                r0 = t * P
                nc.sync.dma_start(xt[:, :q4], x_d[r0 : r0 + P, :q4])
                nc.scalar.dma_start(xt[:, q4 : 2 * q4], x_d[r0 : r0 + P, q4 : 2 * q4])
                nc.gpsimd.dma_start(
                    xt[:, 2 * q4 : 3 * q4], x_d[r0 : r0 + P, 2 * q4 : 3 * q4]
                )
                nc.gpsimd.dma_start(xt[:, 3 * q4 :], x_d[r0 : r0 + P, 3 * q4 :])
                amax = sv_pool.tile([P, 1], f32, tag="amax", name=f"amax_{sfx}")
                nc.vector.reduce_max(
                    amax[:], xt[:], axis=AX.X, apply_absolute_value=True
                )
                amax_c = sv_pool.tile([P, 1], f32, tag="amaxc", name=f"amaxc_{sfx}")
                nc.vector.tensor_scalar(amax_c[:], amax[:], 1e-5, None, op0=OP.max)
                r_amax = sv_pool.tile([P, 1], f32, tag="ramax", name=f"ramax_{sfx}")
                nc.vector.reciprocal(r_amax[:], amax_c[:])
                s_act = sv_pool.tile([P, 1], f32, tag="sact", name=f"sact_{sfx}")
                nc.vector.tensor_scalar(s_act[:], r_amax[:], 127.0, None, op0=OP.mult)
                o_scale = sv_pool.tile([P, 1], f32, tag="oscale", name=f"oscale_{sfx}")
                nc.vector.tensor_scalar(
                    o_scale[:], amax_c[:], 2e-6, s_w127[:], op0=OP.add, op1=OP.mult
                )
                # pass A (ACT, in-place): x*s_act + MAGIC  (rounds to int)
                nc.scalar.activation(
                    xt[:], xt[:], ACTF.Identity, bias=m_ap[:], scale=s_act[:]
                )
                # pass B (GpSimd): subtract MAGIC, cast bf16
                qb = qb_pool.tile([P, D_IN], bf16, tag="qb", name=f"qb_{sfx}")
                nc.gpsimd.tensor_scalar(qb[:], xt[:], MAGIC, None, op0=OP.subtract)
                # transpose q: 32 x [128,128] PE transposes, packed 4/psum bank
                qt = qt_pool.tile([P, KT, P], bf16, tag="qt", name=f"qt_{sfx}")
                psq = None
                for k in range(KT):
                    j = k % TPACK
                    if j == 0:
                        psq = ptr.tile(
                            [P, TPACK, P], bf16, tag="tr", name=f"qtr_{sfx}_{k}"
                        )
                    nc.tensor.transpose(
                        psq[:, j, :], qb[:, k * P : (k + 1) * P], ident_b[:]
                    )
                    if j == TPACK - 1:
                        k0 = k - (TPACK - 1)
                        nc.vector.tensor_copy(qt[:, k0 : k + 1, :], psq[:])
                # matmuls: accumulate over k into 3 psum regions
                accs = [
                    pacc.tile([P, w], f32, tag=f"a{ci}", name=f"acc{ci}_{sfx}")
                    for ci, (off, w) in enumerate(N_CHUNKS)
                ]
                for k in range(KT):
                    st, sp = (k == 0), (k == KT - 1)
                    for ci, (off, w) in enumerate(N_CHUNKS):
                        nc.tensor.matmul(
                            accs[ci][:],
                            qt[:, k, :],
                            twt[:, k, off : off + w],
                            start=st,
                            stop=sp,
                        )
                # evict with per-token scale on ACT (Identity keeps ACT table warm)
                osb = out_pool.tile([P, O_SHARD], f32, tag="osb", name=f"osb_{sfx}")
                for ci, (off, w) in enumerate(N_CHUNKS):
                    nc.scalar.activation(
                        osb[:, off : off + w],
                        accs[ci][:],
                        ACTF.Identity,
                        bias=zero_ap[:],
                        scale=o_scale[:],
                    )
                # output store on SWDGE queues, off the busy SP/ACT rings
                nc.gpsimd.dma_start(out_d[t * P : (t + 1) * P, :], osb[:])

    return nc


def _get_nc():
    global _BUILT
    if _BUILT is None:
        _BUILT = _build()
        _BUILT.finalize()
    return _BUILT


def _run(x, w, trace=False):
    from concourse.bass_utils import run_bass_kernel_spmd

    nc = _get_nc()
    x = np.ascontiguousarray(np.asarray(x, dtype=np.float32))
    w = np.ascontiguousarray(np.asarray(w, dtype=np.float32))
    in_maps = [
        {"x": x, "w": w[i * O_SHARD : (i + 1) * O_SHARD, :]} for i in range(N_CORES)
    ]
    res = run_bass_kernel_spmd(nc, in_maps, core_ids=list(range(N_CORES)), trace=trace)
    out = np.concatenate([res.results[i]["out"] for i in range(N_CORES)], axis=1)
    return out, res


def kernel(x, w):
    out, _ = _run(x, w, trace=False)
    return out


def _make_sharded(nc, n_cores, donate):
    """Replicate bass2jax.run_bass_via_pjrt's shard_map build, optionally
    without output-buffer donation so the compiled fn can be re-run for
    steady-state timing with device-resident inputs."""
    import jax
    import numpy as _np
    from jax.sharding import Mesh, PartitionSpec
    from jax.experimental.shard_map import shard_map
    import concourse.mybir as mybir
    from concourse import bass2jax
    from concourse.bass2jax import _bass_exec_p, install_neuronx_cc_hook

    install_neuronx_cc_hook()

    partition_name = nc.partition_id_tensor.name if nc.partition_id_tensor else None
    in_names, out_names, out_avals, zero_outs = [], [], [], []
    for alloc in nc.m.functions[0].allocations:
        if not isinstance(alloc, mybir.MemoryLocationSet):
            continue
        name = alloc.memorylocations[0].name
        if alloc.kind == "ExternalInput":
            if name != partition_name:
                in_names.append(name)
        elif alloc.kind == "ExternalOutput":
            out_names.append(name)
            shape = tuple(alloc.tensor_shape)
            dtype = mybir.dt.np(alloc.dtype)
            out_avals.append(jax.core.ShapedArray(shape, dtype))
            zero_outs.append(_np.zeros(shape, dtype))
    n_params = len(in_names)
    in_names = in_names + out_names
    if partition_name is not None:
        in_names.append(partition_name)

    def _body(*args):
        operands = list(args)
        if partition_name is not None:
            operands.append(bass2jax.partition_id_tensor())
        outs = _bass_exec_p.bind(
            *operands,
            out_avals=tuple(out_avals),
            in_names=tuple(in_names),
            out_names=tuple(out_names),
            lowering_input_output_aliases=(),
            sim_require_finite=True,
            sim_require_nnan=True,
            nc=nc,
        )
        return tuple(outs)

    devices = jax.devices()[:n_cores]
    mesh = Mesh(_np.asarray(devices), ("core",))
    n_outs = len(out_names)
    in_specs = (PartitionSpec("core"),) * (n_params + n_outs)
    out_specs = (PartitionSpec("core"),) * n_outs
    kw = dict(keep_unused=True)
    if donate:
        kw["donate_argnums"] = tuple(range(n_params, n_params + n_outs))
    sharded = jax.jit(
        shard_map(_body, mesh=mesh, in_specs=in_specs, out_specs=out_specs,
                  check_rep=False),
        **kw,
    )
    from jax.sharding import NamedSharding

    in_sharding = NamedSharding(mesh, PartitionSpec("core"))
    return sharded, in_names[:n_params], out_names, zero_outs, in_sharding


def _make_sharded_chain(nc, n_cores, n_chain):
    """Like _make_sharded but the body executes the NEFF n_chain times
    sequentially (each call's output donated as the next call's out buffer),
    so one host dispatch measures n_chain on-device executions."""
    import jax
    import numpy as _np
    from jax.sharding import Mesh, PartitionSpec, NamedSharding
    from jax.experimental.shard_map import shard_map
    import concourse.mybir as mybir
    from concourse import bass2jax
    from concourse.bass2jax import _bass_exec_p, install_neuronx_cc_hook

    install_neuronx_cc_hook()

    partition_name = nc.partition_id_tensor.name if nc.partition_id_tensor else None
    in_names, out_names, out_avals, zero_outs = [], [], [], []
    for alloc in nc.m.functions[0].allocations:
        if not isinstance(alloc, mybir.MemoryLocationSet):
            continue
        name = alloc.memorylocations[0].name
        if alloc.kind == "ExternalInput":
            if name != partition_name:
                in_names.append(name)
        elif alloc.kind == "ExternalOutput":
            out_names.append(name)
            shape = tuple(alloc.tensor_shape)
            dtype = mybir.dt.np(alloc.dtype)
            out_avals.append(jax.core.ShapedArray(shape, dtype))
            zero_outs.append(_np.zeros(shape, dtype))
    n_params = len(in_names)
    all_in_names = in_names + out_names
    if partition_name is not None:
        all_in_names.append(partition_name)

    def _body(*args):
        params = list(args[:n_params])
        outs = list(args[n_params:])
        for _ in range(n_chain):
            operands = params + outs
            if partition_name is not None:
                operands.append(bass2jax.partition_id_tensor())
            outs = list(
                _bass_exec_p.bind(
                    *operands,
                    out_avals=tuple(out_avals),
                    in_names=tuple(all_in_names),
                    out_names=tuple(out_names),
                    lowering_input_output_aliases=(),
                    sim_require_finite=True,
                    sim_require_nnan=True,
                    nc=nc,
                )
            )
        return tuple(outs)

    devices = jax.devices()[:n_cores]
    mesh = Mesh(_np.asarray(devices), ("core",))
    n_outs = len(out_names)
    in_specs = (PartitionSpec("core"),) * (n_params + n_outs)
    out_specs = (PartitionSpec("core"),) * n_outs
    sharded = jax.jit(
        shard_map(_body, mesh=mesh, in_specs=in_specs, out_specs=out_specs,
                  check_rep=False),
        keep_unused=True,
    )
    in_sharding = NamedSharding(mesh, PartitionSpec("core"))
    return sharded, in_names, out_names, zero_outs, in_sharding


def bench_repeat(x, w, n_repeat=4, iters=8):
    """Per-forward device time: build the kernel with the main loop repeated
    n_repeat times inside one NEFF; (t_M - t_1)/(M-1) cancels dispatch
    overhead and the w-prologue."""
    import time
    import jax

    x = np.ascontiguousarray(np.asarray(x, dtype=np.float32))
    w = np.ascontiguousarray(np.asarray(w, dtype=np.float32))
    in_maps = [
        {"x": x, "w": w[i * O_SHARD : (i + 1) * O_SHARD, :]} for i in range(N_CORES)
    ]
    results = {}
    for rep in (1, n_repeat):
        nc = _build(TT, n_repeat=rep)
        nc.finalize()
        sharded, in_names, out_names, zero_outs, in_sharding = _make_sharded(
            nc, N_CORES, donate=False
        )
        concat_in = [
            np.concatenate([in_maps[c][nm] for c in range(N_CORES)], axis=0)
            for nm in in_names
        ]
        concat_zeros = [
            np.zeros((N_CORES * z.shape[0], *z.shape[1:]), z.dtype) for z in zero_outs
        ]
        args = [jax.device_put(a, in_sharding) for a in concat_in + concat_zeros]
        jax.block_until_ready(args)
        outs = sharded(*args)
        jax.block_until_ready(outs)
        times = []
        for _ in range(iters):
            t0 = time.perf_counter()
            outs = sharded(*args)
            jax.block_until_ready(outs)
            times.append(time.perf_counter() - t0)
        times.sort()
        results[rep] = times
    per_exec = (results[n_repeat][0] - results[1][0]) / (n_repeat - 1)
    return per_exec, results


def bench_chain(x, w, n_chain=8, iters=5, n_ttiles=TT):
    """Per-execution device time via chained in-dispatch executions."""
    import time
    import jax

    if n_ttiles != TT:
        nc = _build(n_ttiles)
        nc.finalize()
    else:
        nc = _get_nc()
    x = np.ascontiguousarray(np.asarray(x, dtype=np.float32))
    w = np.ascontiguousarray(np.asarray(w, dtype=np.float32))
    in_maps = [
        {"x": x[: n_ttiles * P], "w": w[i * O_SHARD : (i + 1) * O_SHARD, :]}
        for i in range(N_CORES)
    ]
    results = {}
    for nch in (1, n_chain):
        sharded, in_names, out_names, zero_outs, in_sharding = _make_sharded_chain(
            nc, N_CORES, nch
        )
        concat_in = [
            np.concatenate([in_maps[c][nm] for c in range(N_CORES)], axis=0)
            for nm in in_names
        ]
        concat_zeros = [
            np.zeros((N_CORES * z.shape[0], *z.shape[1:]), z.dtype) for z in zero_outs
        ]
        args = [jax.device_put(a, in_sharding) for a in concat_in + concat_zeros]
        jax.block_until_ready(args)
        outs = sharded(*args)
        jax.block_until_ready(outs)
        times = []
        for _ in range(iters):
            t0 = time.perf_counter()
            outs = sharded(*args)
            jax.block_until_ready(outs)
            times.append(time.perf_counter() - t0)
        results[nch] = min(times)
    per_exec = (results[n_chain] - results[1]) / (n_chain - 1)
    return per_exec, results


def bench(x, w, iters=10, n_ttiles=TT):
    """Steady-state timing: device-resident inputs, repeated execution."""
    import time
    import jax

    if n_ttiles != TT:
        nc = _build(n_ttiles)
        nc.finalize()
    else:
        nc = _get_nc()
    x = np.ascontiguousarray(np.asarray(x, dtype=np.float32))
    w = np.ascontiguousarray(np.asarray(w, dtype=np.float32))
    in_maps = [
        {"x": x[: n_ttiles * P], "w": w[i * O_SHARD : (i + 1) * O_SHARD, :]}
        for i in range(N_CORES)
    ]
    sharded, in_names, out_names, zero_outs, in_sharding = _make_sharded(
        nc, N_CORES, donate=False
    )
    concat_in = [
        np.concatenate([in_maps[c][nm] for c in range(N_CORES)], axis=0)
        for nm in in_names
    ]
    concat_zeros = [
        np.zeros((N_CORES * z.shape[0], *z.shape[1:]), z.dtype) for z in zero_outs
    ]
    args = [jax.device_put(a, in_sharding) for a in concat_in + concat_zeros]
    jax.block_until_ready(args)
    # warmup (compiles)
    outs = sharded(*args)
    jax.block_until_ready(outs)
    times = []
    for _ in range(iters):
        t0 = time.perf_counter()
        outs = sharded(*args)
        jax.block_until_ready(outs)
        times.append(time.perf_counter() - t0)
    out0 = np.asarray(outs[out_names.index("out")])
    full = np.concatenate(
        [out0.reshape(N_CORES, n_ttiles * P, O_SHARD)[c] for c in range(N_CORES)],
        axis=1,
    )
    return full, times


# revision 31
# speedup vs baseline: 1.0179x; 1.0179x over previous
"""BitLinearx (BitNet-style ternary-weight + int8-activation linear) on 8 TRN2 cores.

Tensor-parallel: shard weight rows (out_features=11008) across 8 cores (1376 each),
replicate activations. Each core computes its [8192, 1376] output slice; host
concatenates. s_w = 1/mean(|w|) uses an on-device AllReduce.

Math notes:
- q = clip(round(x * 127/amax), -128, 127) are exact integers in [-128, 127] and
  tw = clip(round(w*s_w), -1, 1) in {-1, 0, 1}: both exact in bf16, and the
  matmul accumulates integers < 2^24 in fp32 PSUM, so the core matmul is EXACT.
- round-to-nearest-even is implemented as (v + 1.5*2^23) - 1.5*2^23 in fp32.
- For the weights, clip-then-round == round-then-clip (clip bounds are integers),
  which lets the two clip ops fuse with the scale multiply.
- For activations round(x * 127/amax) can never leave [-127, 127] (the max
  |x| element maps to exactly +-127), so the activation clip is a no-op and
  is elided. (Only exception: all |x| <= 1e-5, impossible for randn fill.)
"""

import numpy as np

T = 8192
D_IN = 4096
D_OUT = 11008
N_CORES = 8
O_SHARD = D_OUT // N_CORES  # 1376
P = 128
KT = D_IN // P  # 32 contraction tiles
TT = T // P  # 64 token tiles
OT_FULL = O_SHARD // P  # 10 full o tiles
O_REM = O_SHARD - OT_FULL * P  # 96
MAGIC = 12582912.0  # 1.5 * 2**23, fp32 RNE rounding constant
N_CHUNKS = ((0, 512), (512, 512), (1024, 352))  # psum accumulation regions
TPACK = 4  # transposes packed per psum bank before one copyback

_BUILT = None


def _build(n_ttiles=TT, n_repeat=1):
    import concourse.bacc as bacc
    import concourse.mybir as mybir
    import concourse.tile as tile
    from concourse.masks import make_identity

    f32 = mybir.dt.float32
    bf16 = mybir.dt.bfloat16
    AX = mybir.AxisListType
    OP = mybir.AluOpType
    ACTF = mybir.ActivationFunctionType

    # Bacc (not Bass): its finalize() runs generate_event_semaphores /
    # move_matmul_waits_to_ldweights, legalizing multi-wait instructions for
    # walrus (which allows only one sync wait per instruction).
    nc = bacc.Bacc("TRN2", num_devices=N_CORES, num_swdge_queues=4)

    t_rows = n_ttiles * P
    x_d = nc.dram_tensor("x", [t_rows, D_IN], f32, kind="ExternalInput")
    w_d = nc.dram_tensor("w", [O_SHARD, D_IN], f32, kind="ExternalInput")
    out_d = nc.dram_tensor("out", [t_rows, O_SHARD], f32, kind="ExternalOutput")
    cc_in = nc.dram_tensor("cc_in", [P, 1], f32)
    cc_out = nc.dram_tensor("cc_out", [P, 1], f32, addr_space="Shared")

    with tile.TileContext(nc) as tc:
        with (
            tc.tile_pool(name="xw", bufs=3) as xw_pool,  # [128,4096] f32 x/w tiles
            tc.tile_pool(name="qb", bufs=2) as qb_pool,  # [128,4096] bf16 q natural
            tc.tile_pool(name="qt", bufs=2) as qt_pool,  # [128,32,128] bf16 qT
            tc.tile_pool(name="twt", bufs=1) as twt_pool,  # [128,32,1376] bf16 twT
            tc.tile_pool(name="osb", bufs=2) as out_pool,  # [128,1376] f32 out stage
            tc.tile_pool(name="const", bufs=1) as const_pool,
            tc.tile_pool(name="sv", bufs=3) as sv_pool,  # per-tile scalars
            tc.tile_pool(name="pacc", bufs=2, space="PSUM") as pacc,
            tc.tile_pool(name="ptr", bufs=2, space="PSUM") as ptr,
        ):
            # ---------------- constants ----------------
            ones = const_pool.tile([P, P], f32, name="ones")
            nc.gpsimd.memset(ones[:], 1.0)
            ident_b = const_pool.tile([P, P], bf16, name="ident_b")
            make_identity(nc, ident_b[:])
            ident_f = const_pool.tile([P, P], f32, name="ident_f")
            make_identity(nc, ident_f[:])
            m_ap = const_pool.tile([P, 1], f32, name="m_ap")
            nc.gpsimd.memset(m_ap[:], MAGIC)
            zero_ap = const_pool.tile([P, 1], f32, name="zero_ap")
            nc.gpsimd.memset(zero_ap[:], 0.0)

            # ---------------- phase W1: sum(|w|) partials + AllReduce ----------------
            n_wt = OT_FULL + 1
            parts = const_pool.tile([P, n_wt], f32, name="parts")
            nc.vector.memset(parts[:], 0.0)
            for i in range(n_wt):
                rows = P if i < OT_FULL else O_REM
                wt = xw_pool.tile([P, D_IN], f32, tag="xw", name=f"w1_{i}")
                q4 = D_IN // 4
                nc.sync.dma_start(wt[:rows, :q4], w_d[i * P : i * P + rows, :q4])
                nc.scalar.dma_start(
                    wt[:rows, q4 : 2 * q4], w_d[i * P : i * P + rows, q4 : 2 * q4]
                )
                nc.gpsimd.dma_start(
                    wt[:rows, 2 * q4 : 3 * q4],
                    w_d[i * P : i * P + rows, 2 * q4 : 3 * q4],
                )
                nc.gpsimd.dma_start(
                    wt[:rows, 3 * q4 :], w_d[i * P : i * P + rows, 3 * q4 :]
                )
                nc.vector.reduce_sum(
                    parts[:rows, i : i + 1],
                    wt[:rows, :],
                    axis=AX.X,
                    apply_absolute_value=True,
                )
            acc_sum = const_pool.tile([P, 1], f32, name="acc_sum")
            nc.vector.reduce_sum(acc_sum[:], parts[:], axis=AX.X)
            nc.sync.dma_start(cc_in[:], acc_sum[:])
            nc.gpsimd.collective_compute(
                "AllReduce",
                OP.add,
                replica_groups=[list(range(N_CORES))],
                ins=[cc_in[:]],
                outs=[cc_out[:]],
            )
            allred_sb = const_pool.tile([P, 1], f32, name="allred_sb")
            nc.sync.dma_start(allred_sb[:], cc_out[:])

            # cross-partition sum + broadcast in one matmul: psum[m,0]=sum_p allred[p]
            gsum_ps = ptr.tile([P, 1], f32, tag="tr", name="gsum_ps")
            nc.tensor.matmul(gsum_ps[:], ones[:], allred_sb[:], start=True, stop=True)
            mean_c = const_pool.tile([P, 1], f32, name="mean_c")
            nc.vector.tensor_scalar(
                mean_c[:],
                gsum_ps[:],
                1.0 / float(D_OUT * D_IN),
                1e-5,
                op0=OP.mult,
                op1=OP.max,
            )
            s_w = const_pool.tile([P, 1], f32, name="s_w")
            nc.vector.reciprocal(s_w[:], mean_c[:])
            s_w127 = const_pool.tile([P, 1], f32, name="s_w127")
            nc.vector.tensor_scalar(s_w127[:], s_w[:], 1.0 / 127.0, None, op0=OP.mult)

            # ---------------- phase W2: ternarize + transpose w ----------------
            twt = twt_pool.tile([P, KT, O_SHARD], bf16, name="twt")
            for i in range(OT_FULL + 1):
                rows = P if i < OT_FULL else O_REM
                wt = xw_pool.tile([P, D_IN], f32, tag="xw", name=f"w2_{i}")
                q4 = D_IN // 4
                nc.sync.dma_start(wt[:rows, :q4], w_d[i * P : i * P + rows, :q4])
                nc.scalar.dma_start(
                    wt[:rows, q4 : 2 * q4], w_d[i * P : i * P + rows, q4 : 2 * q4]
                )
                nc.gpsimd.dma_start(
                    wt[:rows, 2 * q4 : 3 * q4],
                    w_d[i * P : i * P + rows, 2 * q4 : 3 * q4],
                )
                nc.gpsimd.dma_start(
                    wt[:rows, 3 * q4 :], w_d[i * P : i * P + rows, 3 * q4 :]
                )
                # clamp(w*s_w, -1, 1) then +MAGIC (rounds): two fused in-place passes
                nc.vector.tensor_scalar(
                    wt[:rows, :], wt[:rows, :], s_w[:rows, :], 1.0,
                    op0=OP.mult, op1=OP.min,
                )
                nc.vector.tensor_scalar(
                    wt[:rows, :], wt[:rows, :], -1.0, MAGIC,
                    op0=OP.max, op1=OP.add,
                )
                pst = None
                for k in range(KT):
                    j = k % TPACK
                    if j == 0:
                        pst = ptr.tile(
                            [P, TPACK, P], f32, tag="tr", name=f"wtr_{i}_{k}"
                        )
                    nc.tensor.transpose(
                        pst[:, j, :rows],
                        wt[:rows, k * P : (k + 1) * P],
                        ident_f[:rows, :rows],
                    )
                    if j == TPACK - 1:
                        k0 = k - (TPACK - 1)
                        nc.vector.tensor_scalar(
                            twt[:, k0 : k + 1, i * P : i * P + rows],
                            pst[:, :, :rows],
                            MAGIC,
                            None,
                            op0=OP.subtract,
                        )

            # ---------------- main loop over token tiles ----------------
            for rep, t in ((r, t) for r in range(n_repeat) for t in range(n_ttiles)):
                t = t if rep == 0 else t  # same data each repeat (timing builds)
                sfx = f"{t}" if n_repeat == 1 else f"{rep}_{t}"
                xt = xw_pool.tile([P, D_IN], f32, tag="xw", name=f"x_{sfx}")
                # Spread the 2MB tile load across all DMA paths: each HWDGE
                # ring streams ~22.5GB/s, so one dma_start (89us) would starve
                # the 21us/tile PE pipeline. SP + ACT rings take a quarter
                # each, SWDGE (8 queues) takes the rest.
                q4 = D_IN // 4
                r0 = t * P
                nc.sync.dma_start(xt[:, :q4], x_d[r0 : r0 + P, :q4])
                nc.scalar.dma_start(xt[:, q4 : 2 * q4], x_d[r0 : r0 + P, q4 : 2 * q4])
                nc.gpsimd.dma_start(
                    xt[:, 2 * q4 : 3 * q4], x_d[r0 : r0 + P, 2 * q4 : 3 * q4]
                )
                nc.gpsimd.dma_start(xt[:, 3 * q4 :], x_d[r0 : r0 + P, 3 * q4 :])
                amax = sv_pool.tile([P, 1], f32, tag="amax", name=f"amax_{sfx}")
                nc.vector.reduce_max(
                    amax[:], xt[:], axis=AX.X, apply_absolute_value=True
                )
                amax_c = sv_pool.tile([P, 1], f32, tag="amaxc", name=f"amaxc_{sfx}")
                nc.vector.tensor_scalar(amax_c[:], amax[:], 1e-5, None, op0=OP.max)
                r_amax = sv_pool.tile([P, 1], f32, tag="ramax", name=f"ramax_{sfx}")
                nc.vector.reciprocal(r_amax[:], amax_c[:])
                s_act = sv_pool.tile([P, 1], f32, tag="sact", name=f"sact_{sfx}")
                nc.vector.tensor_scalar(s_act[:], r_amax[:], 127.0, None, op0=OP.mult)
                o_scale = sv_pool.tile([P, 1], f32, tag="oscale", name=f"oscale_{sfx}")
                nc.vector.tensor_scalar(
                    o_scale[:], amax_c[:], 2e-6, s_w127[:], op0=OP.add, op1=OP.mult
                )
                # pass A (DVE, in-place): x*s_act + MAGIC  (rounds to int).
                # On DVE, not ACT: ACT's in-order queue holds the psum-evict of
                # the PREVIOUS tile, which waits for its matmuls to finish —
                # putting passA there stalls the next tile's whole quant chain
                # and idles the PE every iteration.
                nc.vector.tensor_scalar(
                    xt[:], xt[:], s_act[:], MAGIC, op0=OP.mult, op1=OP.add
                )
                # pass B (GpSimd): subtract MAGIC, cast bf16
                qb = qb_pool.tile([P, D_IN], bf16, tag="qb", name=f"qb_{sfx}")
                nc.gpsimd.tensor_scalar(qb[:], xt[:], MAGIC, None, op0=OP.subtract)
                # transpose q: 32 x [128,128] PE transposes, packed 4/psum bank
                qt = qt_pool.tile([P, KT, P], bf16, tag="qt", name=f"qt_{sfx}")
                psq = None
                for k in range(KT):
                    j = k % TPACK
                    if j == 0:
                        psq = ptr.tile(
                            [P, TPACK, P], bf16, tag="tr", name=f"qtr_{sfx}_{k}"
                        )
                    nc.tensor.transpose(
                        psq[:, j, :], qb[:, k * P : (k + 1) * P], ident_b[:]
                    )
                    if j == TPACK - 1:
                        k0 = k - (TPACK - 1)
                        nc.vector.tensor_copy(qt[:, k0 : k + 1, :], psq[:])
                # matmuls: accumulate over k into 3 psum regions
                accs = [
                    pacc.tile([P, w], f32, tag=f"a{ci}", name=f"acc{ci}_{sfx}")
                    for ci, (off, w) in enumerate(N_CHUNKS)
                ]
                for k in range(KT):
                    st, sp = (k == 0), (k == KT - 1)
                    for ci, (off, w) in enumerate(N_CHUNKS):
                        nc.tensor.matmul(
                            accs[ci][:],
                            qt[:, k, :],
                            twt[:, k, off : off + w],
                            start=st,
                            stop=sp,
                        )
                # evict with per-token scale on ACT (Identity keeps ACT table warm)
                osb = out_pool.tile([P, O_SHARD], f32, tag="osb", name=f"osb_{sfx}")
                for ci, (off, w) in enumerate(N_CHUNKS):
                    nc.scalar.activation(
                        osb[:, off : off + w],
                        accs[ci][:],
                        ACTF.Identity,
                        bias=zero_ap[:],
                        scale=o_scale[:],
                    )
                # output store on SWDGE queues, off the busy SP/ACT rings
                nc.gpsimd.dma_start(out_d[t * P : (t + 1) * P, :], osb[:])

    return nc


def _get_nc():
    global _BUILT
    if _BUILT is None:
        _BUILT = _build()
        _BUILT.finalize()
    return _BUILT


def _run(x, w, trace=False):
    from concourse.bass_utils import run_bass_kernel_spmd

    nc = _get_nc()
    x = np.ascontiguousarray(np.asarray(x, dtype=np.float32))
    w = np.ascontiguousarray(np.asarray(w, dtype=np.float32))
    in_maps = [
        {"x": x, "w": w[i * O_SHARD : (i + 1) * O_SHARD, :]} for i in range(N_CORES)
    ]
    res = run_bass_kernel_spmd(nc, in_maps, core_ids=list(range(N_CORES)), trace=trace)
    out = np.concatenate([res.results[i]["out"] for i in range(N_CORES)], axis=1)
    return out, res


def kernel(x, w):
    out, _ = _run(x, w, trace=False)
    return out


def _make_sharded(nc, n_cores, donate):
    """Replicate bass2jax.run_bass_via_pjrt's shard_map build, optionally
    without output-buffer donation so the compiled fn can be re-run for
    steady-state timing with device-resident inputs."""
    import jax
    import numpy as _np
    from jax.sharding import Mesh, PartitionSpec
    from jax.experimental.shard_map import shard_map
    import concourse.mybir as mybir
    from concourse import bass2jax
    from concourse.bass2jax import _bass_exec_p, install_neuronx_cc_hook

    install_neuronx_cc_hook()

    partition_name = nc.partition_id_tensor.name if nc.partition_id_tensor else None
    in_names, out_names, out_avals, zero_outs = [], [], [], []
    for alloc in nc.m.functions[0].allocations:
        if not isinstance(alloc, mybir.MemoryLocationSet):
            continue
        name = alloc.memorylocations[0].name
        if alloc.kind == "ExternalInput":
            if name != partition_name:
                in_names.append(name)
        elif alloc.kind == "ExternalOutput":
            out_names.append(name)
            shape = tuple(alloc.tensor_shape)
            dtype = mybir.dt.np(alloc.dtype)
            out_avals.append(jax.core.ShapedArray(shape, dtype))
            zero_outs.append(_np.zeros(shape, dtype))
    n_params = len(in_names)
    in_names = in_names + out_names
    if partition_name is not None:
        in_names.append(partition_name)

    def _body(*args):
        operands = list(args)
        if partition_name is not None:
            operands.append(bass2jax.partition_id_tensor())
        outs = _bass_exec_p.bind(
            *operands,
            out_avals=tuple(out_avals),
            in_names=tuple(in_names),
            out_names=tuple(out_names),
            lowering_input_output_aliases=(),
            sim_require_finite=True,
            sim_require_nnan=True,
            nc=nc,
        )
        return tuple(outs)

    devices = jax.devices()[:n_cores]
    mesh = Mesh(_np.asarray(devices), ("core",))
    n_outs = len(out_names)
    in_specs = (PartitionSpec("core"),) * (n_params + n_outs)
    out_specs = (PartitionSpec("core"),) * n_outs
    kw = dict(keep_unused=True)
    if donate:
        kw["donate_argnums"] = tuple(range(n_params, n_params + n_outs))
    sharded = jax.jit(
        shard_map(_body, mesh=mesh, in_specs=in_specs, out_specs=out_specs,
                  check_rep=False),
        **kw,
    )
    from jax.sharding import NamedSharding

    in_sharding = NamedSharding(mesh, PartitionSpec("core"))
    return sharded, in_names[:n_params], out_names, zero_outs, in_sharding


def _make_sharded_chain(nc, n_cores, n_chain):
    """Like _make_sharded but the body executes the NEFF n_chain times
    sequentially (each call's output donated as the next call's out buffer),
    so one host dispatch measures n_chain on-device executions."""
    import jax
    import numpy as _np
    from jax.sharding import Mesh, PartitionSpec, NamedSharding
    from jax.experimental.shard_map import shard_map
    import concourse.mybir as mybir
    from concourse import bass2jax
    from concourse.bass2jax import _bass_exec_p, install_neuronx_cc_hook

    install_neuronx_cc_hook()

    partition_name = nc.partition_id_tensor.name if nc.partition_id_tensor else None
    in_names, out_names, out_avals, zero_outs = [], [], [], []
    for alloc in nc.m.functions[0].allocations:
        if not isinstance(alloc, mybir.MemoryLocationSet):
            continue
        name = alloc.memorylocations[0].name
        if alloc.kind == "ExternalInput":
            if name != partition_name:
                in_names.append(name)
        elif alloc.kind == "ExternalOutput":
            out_names.append(name)
            shape = tuple(alloc.tensor_shape)
            dtype = mybir.dt.np(alloc.dtype)
            out_avals.append(jax.core.ShapedArray(shape, dtype))
            zero_outs.append(_np.zeros(shape, dtype))
    n_params = len(in_names)
    all_in_names = in_names + out_names
    if partition_name is not None:
        all_in_names.append(partition_name)

    def _body(*args):
        params = list(args[:n_params])
        outs = list(args[n_params:])
        for _ in range(n_chain):
            operands = params + outs
            if partition_name is not None:
                operands.append(bass2jax.partition_id_tensor())
            outs = list(
                _bass_exec_p.bind(
                    *operands,
                    out_avals=tuple(out_avals),
                    in_names=tuple(all_in_names),
                    out_names=tuple(out_names),
                    lowering_input_output_aliases=(),
                    sim_require_finite=True,
                    sim_require_nnan=True,
                    nc=nc,
                )
            )
        return tuple(outs)

    devices = jax.devices()[:n_cores]
    mesh = Mesh(_np.asarray(devices), ("core",))
    n_outs = len(out_names)
    in_specs = (PartitionSpec("core"),) * (n_params + n_outs)
    out_specs = (PartitionSpec("core"),) * n_outs
    sharded = jax.jit(
        shard_map(_body, mesh=mesh, in_specs=in_specs, out_specs=out_specs,
                  check_rep=False),
        keep_unused=True,
    )
    in_sharding = NamedSharding(mesh, PartitionSpec("core"))
    return sharded, in_names, out_names, zero_outs, in_sharding


def bench_repeat(x, w, n_repeat=4, iters=8):
    """Per-forward device time: build the kernel with the main loop repeated
    n_repeat times inside one NEFF; (t_M - t_1)/(M-1) cancels dispatch
    overhead and the w-prologue."""
    import time
    import jax

    x = np.ascontiguousarray(np.asarray(x, dtype=np.float32))
    w = np.ascontiguousarray(np.asarray(w, dtype=np.float32))
    in_maps = [
        {"x": x, "w": w[i * O_SHARD : (i + 1) * O_SHARD, :]} for i in range(N_CORES)
    ]
    results = {}
    for rep in (1, n_repeat):
        nc = _build(TT, n_repeat=rep)
        nc.finalize()
        sharded, in_names, out_names, zero_outs, in_sharding = _make_sharded(
            nc, N_CORES, donate=False
        )
        concat_in = [
            np.concatenate([in_maps[c][nm] for c in range(N_CORES)], axis=0)
            for nm in in_names
        ]
        concat_zeros = [
            np.zeros((N_CORES * z.shape[0], *z.shape[1:]), z.dtype) for z in zero_outs
        ]
        args = [jax.device_put(a, in_sharding) for a in concat_in + concat_zeros]
        jax.block_until_ready(args)
        outs = sharded(*args)
        jax.block_until_ready(outs)
        times = []
        for _ in range(iters):
            t0 = time.perf_counter()
            outs = sharded(*args)
            jax.block_until_ready(outs)
            times.append(time.perf_counter() - t0)
        times.sort()
        results[rep] = times
    per_exec = (results[n_repeat][0] - results[1][0]) / (n_repeat - 1)
    return per_exec, results


def bench_chain(x, w, n_chain=8, iters=5, n_ttiles=TT):
    """Per-execution device time via chained in-dispatch executions."""
    import time
    import jax

    if n_ttiles != TT:
        nc = _build(n_ttiles)
        nc.finalize()
    else:
        nc = _get_nc()
    x = np.ascontiguousarray(np.asarray(x, dtype=np.float32))
    w = np.ascontiguousarray(np.asarray(w, dtype=np.float32))
    in_maps = [
        {"x": x[: n_ttiles * P], "w": w[i * O_SHARD : (i + 1) * O_SHARD, :]}
        for i in range(N_CORES)
    ]
    results = {}
    for nch in (1, n_chain):
        sharded, in_names, out_names, zero_outs, in_sharding = _make_sharded_chain(
            nc, N_CORES, nch
        )
        concat_in = [
            np.concatenate([in_maps[c][nm] for c in range(N_CORES)], axis=0)
            for nm in in_names
        ]
        concat_zeros = [
            np.zeros((N_CORES * z.shape[0], *z.shape[1:]), z.dtype) for z in zero_outs
        ]
        args = [jax.device_put(a, in_sharding) for a in concat_in + concat_zeros]
        jax.block_until_ready(args)
        outs = sharded(*args)
        jax.block_until_ready(outs)
        times = []
        for _ in range(iters):
            t0 = time.perf_counter()
            outs = sharded(*args)
            jax.block_until_ready(outs)
            times.append(time.perf_counter() - t0)
        results[nch] = min(times)
    per_exec = (results[n_chain] - results[1]) / (n_chain - 1)
    return per_exec, results


def bench(x, w, iters=10, n_ttiles=TT):
    """Steady-state timing: device-resident inputs, repeated execution."""
    import time
    import jax

    if n_ttiles != TT:
        nc = _build(n_ttiles)
        nc.finalize()
    else:
        nc = _get_nc()
    x = np.ascontiguousarray(np.asarray(x, dtype=np.float32))
    w = np.ascontiguousarray(np.asarray(w, dtype=np.float32))
    in_maps = [
        {"x": x[: n_ttiles * P], "w": w[i * O_SHARD : (i + 1) * O_SHARD, :]}
        for i in range(N_CORES)
    ]
    sharded, in_names, out_names, zero_outs, in_sharding = _make_sharded(
        nc, N_CORES, donate=False
    )
    concat_in = [
        np.concatenate([in_maps[c][nm] for c in range(N_CORES)], axis=0)
        for nm in in_names
    ]
    concat_zeros = [
        np.zeros((N_CORES * z.shape[0], *z.shape[1:]), z.dtype) for z in zero_outs
    ]
    args = [jax.device_put(a, in_sharding) for a in concat_in + concat_zeros]
    jax.block_until_ready(args)
    # warmup (compiles)
    outs = sharded(*args)
    jax.block_until_ready(outs)
    times = []
    for _ in range(iters):
        t0 = time.perf_counter()
        outs = sharded(*args)
        jax.block_until_ready(outs)
        times.append(time.perf_counter() - t0)
    out0 = np.asarray(outs[out_names.index("out")])
    full = np.concatenate(
        [out0.reshape(N_CORES, n_ttiles * P, O_SHARD)[c] for c in range(N_CORES)],
        axis=1,
    )
    return full, times
